# revision 17
# baseline (speedup 1.0000x reference)
"""Causal self-attention (B=4, T=2048, C=768, H=12) on 8 trn2 NeuronCores.

Sharding: core = (batch b in 0..3) x (head-group g in 0..1, 6 heads each).
Each core: QKV projection for its 6 heads, causal attention, partial output
projection (its heads' rows of W_proj). Host sums the two partials per batch
and adds b_proj.

Device-side layout (per core):
  xT [768, 2048]  (host pre-transposes x[b])
  qT/kT produced as [d, t] pair tiles (lhsT = W-slice, rhs = xT)
  v produced natural [t, d] with a ones column appended per head
  S^T [k, q] = kT_block.T @ qT  -> exp on ACT -> PV: y^T += v_aug.T @ expS
    row 64 of the PV accumulator = softmax denominator (ones-column trick)
  normalize via DVE reciprocal + gpsimd partition_broadcast + DVE multiply
  out_partial[t, :] = sum_h yT_h.T @ Wp_h   (y^T is directly the proj lhsT)

Matmul operands are stored bf16 (full PE rate, half the HBM traffic);
accumulation is fp32 in PSUM; the output partials are fp32.
"""

import sys

for _p in ("/opt/pypackages", "/opt/trn_rl_repo"):
    if _p not in sys.path:
        sys.path.insert(0, _p)

import numpy as np
import ml_dtypes

import concourse.bass as bass
import concourse.tile as tile
from concourse import bacc, mybir
from concourse.bass_utils import run_bass_kernel_spmd

B, T, C, H = 4, 2048, 768, 12
HS = C // H            # 64 head dim
HPC = 6                # heads per core
GC = HPC * HS          # 384 columns per core
NCORES = 8
NK = C // 128          # 6 contraction tiles over c_in
P = 128
F32 = mybir.dt.float32
MM = mybir.dt.bfloat16   # matmul operand dtype
NP_MM = ml_dtypes.bfloat16

NQCH = T // 512        # 4 q-chunks of 512
NTB = T // P           # 16 token blocks of 128


def _build_nc():
    nc = bacc.Bacc("TRN2")

    xT = nc.declare_dram_parameter("xT", [C, T], MM, isOutput=False)
    wq = nc.declare_dram_parameter("wq", [C, GC], MM, isOutput=False)
    wk = nc.declare_dram_parameter("wk", [C, GC], MM, isOutput=False)
    wv = nc.declare_dram_parameter("wv", [C, GC], MM, isOutput=False)
    wp = nc.declare_dram_parameter("wp", [GC, C], MM, isOutput=False)
    bqk = nc.declare_dram_parameter("bqk", [P, 6], F32, isOutput=False)
    bv = nc.declare_dram_parameter("bv", [1, GC], F32, isOutput=False)
    mask = nc.declare_dram_parameter("mask", [P, P], MM, isOutput=False)
    out = nc.declare_dram_parameter("out", [T, C], F32, isOutput=True)

    xTv = xT.ap().rearrange("(k p) t -> k p t", p=P)
    wqv = wq.ap().rearrange("(k p) d -> k p d", p=P)
    wkv = wk.ap().rearrange("(k p) d -> k p d", p=P)
    wvv = wv.ap().rearrange("(k p) d -> k p d", p=P)
    wpv = wp.ap().rearrange("(h p) n -> h p n", p=HS)
    outv = out.ap().rearrange("(b p) n -> b p n", p=P)

    with tile.TileContext(nc) as tc:
        from contextlib import ExitStack

        with ExitStack() as ctx:
            pers = ctx.enter_context(tc.tile_pool(name="pers", bufs=1))
            # PSUM: psMM 3 x [128,1024] (2 banks each) + psY 2 x 1 bank = 8 banks
            psMM = ctx.enter_context(tc.tile_pool(name="psMM", bufs=3, space="PSUM"))
            psY = ctx.enter_context(tc.tile_pool(name="psY", bufs=2, space="PSUM"))

            # ---- persistent tiles ----
            qkT = [pers.tile([P, T], MM, name=f"qkT{i}") for i in range(6)]
            vsb = [pers.tile([P, HPC * (HS + 1)], MM, name=f"v{tb}") for tb in range(NTB)]
            wph = [pers.tile([HS, C], MM, name=f"wp{h}") for h in range(HPC)]
            mask_sb = pers.tile([P, P], MM, name="mask")
            bqk_sb = pers.tile([P, 6], F32, name="bqk")
            bv_sb = pers.tile([1, GC], F32, name="bv")
            bvb = pers.tile([P, GC], F32, name="bvb")
            # ones rows for PE-based partition broadcasts (K=1 matmuls)
            ones128 = pers.tile([1, P], MM, name="ones128")
            ones65 = pers.tile([HS + 1, HS], MM, name="ones65")
            bv_bf = pers.tile([1, GC], MM, name="bv_bf")

            nc.sync.dma_start(mask_sb[:], mask.ap())
            nc.sync.dma_start(bqk_sb[:], bqk.ap())
            nc.sync.dma_start(bv_sb[:], bv.ap())
            for h in range(HPC):
                nc.sync.dma_start(wph[h][:], wpv[h])
            nc.vector.memset(ones128[:], 1.0)
            nc.vector.memset(ones65[:], 1.0)
            # bvb[p, :] = bv for all p  (PE broadcast through PSUM); stage bv
            # through a DVE copy so the matmul's producers share one sem
            with nc.allow_low_precision(reason="bias broadcast staging in bf16"):
                nc.vector.tensor_copy(out=bv_bf[:], in_=bv_sb[:])
            ps_b = psMM.tile([P, 1024], F32, tag="mm", name="ps_b")
            nc.tensor.matmul(ps_b[:, 0:GC], ones128[:], bv_bf[:],
                             start=True, stop=True)
            nc.vector.tensor_copy(out=bvb[:], in_=ps_b[:, 0:GC])

            # ---- QKV (xT and W tiles live only for this phase) ----
            with tc.tile_pool(name="wx", bufs=1) as wx:
                xt = [wx.tile([P, T], MM, name=f"xt{k}") for k in range(NK)]
                wqt = [wx.tile([P, GC], MM, name=f"wq{k}") for k in range(NK)]
                wkt = [wx.tile([P, GC], MM, name=f"wk{k}") for k in range(NK)]
                wvt = [wx.tile([P, GC], MM, name=f"wv{k}") for k in range(NK)]
                for k in range(NK):
                    nc.sync.dma_start(xt[k][:], xTv[k])
                    nc.sync.dma_start(wqt[k][:], wqv[k])
                    nc.sync.dma_start(wkt[k][:], wkv[k])
                    nc.sync.dma_start(wvt[k][:], wvv[k])

                # qT / kT pair tiles: i in 0..2 -> q pair i; 3..5 -> k pair i-3
                for i in range(6):
                    wt = wqt if i < 3 else wkt
                    p = i % 3
                    for nch in range(NQCH):
                        ps = psMM.tile([P, 1024], F32, tag="mm", name="ps_qk")
                        for k in range(NK):
                            nc.tensor.matmul(
                                ps[:, 0:512],
                                wt[k][:, P * p:P * (p + 1)],
                                xt[k][:, 512 * nch:512 * (nch + 1)],
                                start=(k == 0),
                                stop=(k == NK - 1),
                            )
                        nc.vector.tensor_scalar_add(
                            out=qkT[i][:, 512 * nch:512 * (nch + 1)],
                            in0=ps[:, 0:512],
                            scalar1=bqk_sb[:, i:i + 1],
                        )

                # v natural [t, d] + bias, plus ones column per head
                for tb in range(NTB):
                    ps = psMM.tile([P, 1024], F32, tag="mm", name="ps_v")
                    for k in range(NK):
                        nc.tensor.matmul(
                            ps[:, 0:GC],
                            xt[k][:, P * tb:P * (tb + 1)],
                            wvt[k][:],
                            start=(k == 0),
                            stop=(k == NK - 1),
                        )
                    v3 = vsb[tb].rearrange("p (h e) -> p h e", e=HS + 1)
                    nc.vector.tensor_add(
                        out=v3[:, :, 0:HS],
                        in0=ps[:, 0:GC].rearrange("p (h d) -> p h d", d=HS),
                        in1=bvb.rearrange("p (h d) -> p h d", d=HS),
                    )
                    nc.vector.memset(v3[:, :, HS:HS + 1], 1.0)

            # ---- attention + projection, chunk-outer ----
            work = ctx.enter_context(tc.tile_pool(name="work", bufs=3))
            ytp = ctx.enter_context(tc.tile_pool(name="ytp", bufs=2))

            for c in range(NQCH):
                ytiles = []
                for h in range(HPC):
                    hp, hd = divmod(h, 2)
                    qTh = qkT[hp][HS * hd:HS * (hd + 1), :]
                    kTh = qkT[3 + hp][HS * hd:HS * (hd + 1), :]
                    vh = [
                        vsb[j].rearrange("p (h e) -> p h e", e=HS + 1)[:, h, :]
                        for j in range(4 * c + 4)
                    ]

                    yps = psY.tile([HS + 1, 512], F32, tag="y", name="yps")
                    jlast = 4 * c + 3
                    for jp in range(2 * c + 2):
                        sps = psMM.tile([P, 1024], F32, tag="mm", name="ps_s")
                        es = work.tile([P, 1024], MM, tag="es", name="es")
                        for u in range(2):
                            j = 2 * jp + u
                            m = j - 4 * c
                            qs = P * m if m > 0 else 0
                            nc.tensor.matmul(
                                sps[:, 512 * u + qs:512 * (u + 1)],
                                kTh[:, P * j:P * (j + 1)],
                                qTh[:, 512 * c + qs:512 * (c + 1)],
                                start=True,
                                stop=True,
                            )
                        if 2 * jp + 1 - 4 * c > 0:
                            # pair contains diagonal blocks: exp only the
                            # initialized subrange of each half
                            for u in range(2):
                                j = 2 * jp + u
                                m = j - 4 * c
                                qs = P * m if m > 0 else 0
                                nc.scalar.activation(
                                    out=es[:, 512 * u + qs:512 * (u + 1)],
                                    in_=sps[:, 512 * u + qs:512 * (u + 1)],
                                    func=mybir.ActivationFunctionType.Exp,
                                    scale=1.0 / 8.0,
                                )
                        else:
                            nc.scalar.activation(
                                out=es[:],
                                in_=sps[:],
                                func=mybir.ActivationFunctionType.Exp,
                                scale=1.0 / 8.0,
                            )
                        for u in range(2):
                            j = 2 * jp + u
                            m = j - 4 * c
                            qs = P * m if m > 0 else 0
                            if m >= 0:
                                # mask the 128x128 diagonal sub-block
                                nc.vector.tensor_mul(
                                    out=es[:, 512 * u + qs:512 * u + qs + P],
                                    in0=es[:, 512 * u + qs:512 * u + qs + P],
                                    in1=mask_sb[:],
                                )
                            nc.tensor.matmul(
                                yps[:, qs:512],
                                vh[j],
                                es[:, 512 * u + qs:512 * (u + 1)],
                                start=(j == 0),
                                stop=(j == jlast),
                            )

                    # normalize: row HS of yps holds sum(exp)
                    rt = work.tile([HS + 1, 512], MM, tag="rt", name="rt")
                    with nc.allow_low_precision(reason="softmax denom bf16 for full-rate PE broadcast"):
                        nc.vector.reciprocal(out=rt[HS:HS + 1, :], in_=yps[HS:HS + 1, :])
                    # broadcast 1/sum across the 64 head-dim partitions via a
                    # K=1 PE matmul (ones column at partition HS)
                    rb = psMM.tile([P, 1024], F32, tag="mm", name="rb")
                    nc.tensor.matmul(rb[0:HS, 0:512], ones65[HS:HS + 1, :],
                                     rt[HS:HS + 1, :], start=True, stop=True)
                    rb_sb = work.tile([HS, 512], F32, tag="rb", name="rb_sb")
                    nc.vector.tensor_copy(out=rb_sb[:], in_=rb[0:HS, 0:512])
                    yt = ytp.tile([HS, 512], MM, tag=f"yt{h}", name=f"yt{h}")
                    nc.vector.tensor_mul(out=yt[:], in0=yps[0:HS, :], in1=rb_sb[:])
                    ytiles.append(yt)

                # projection for the 4 token-blocks of this chunk
                for tq in range(4):
                    tb = 4 * c + tq
                    pps = psMM.tile([P, 1024], F32, tag="mm", name="ps_o")
                    for h in range(HPC):
                        for n0, nn in ((0, 512), (512, 256)):
                            nc.tensor.matmul(
                                pps[:, n0:n0 + nn],
                                ytiles[h][:, P * tq:P * (tq + 1)],
                                wph[h][:, n0:n0 + nn],
                                start=(h == 0),
                                stop=(h == HPC - 1),
                            )
                    ot = work.tile([P, C], F32, tag="ot", name="ot")
                    nc.vector.tensor_copy(out=ot[:], in_=pps[:, 0:C])
                    nc.sync.dma_start(outv[tb], ot[:])

    nc.compile()
    return nc


_nc_cache = None
last_results = None


def _get_nc():
    global _nc_cache
    if _nc_cache is None:
        _nc_cache = _build_nc()
    return _nc_cache


def make_in_maps(x, W_attn, b_attn, W_proj):
    x = np.asarray(x, np.float32)
    W_attn = np.asarray(W_attn, np.float32)
    b_attn = np.asarray(b_attn, np.float32)
    W_proj = np.asarray(W_proj, np.float32)

    kk, qq = np.meshgrid(np.arange(P), np.arange(P), indexing="ij")
    mask = (qq >= kk).astype(NP_MM)

    in_maps = []
    for core in range(NCORES):
        b, g = divmod(core, 2)
        hs = slice(GC * g, GC * (g + 1))
        bq = b_attn[0:C][hs]
        bk = b_attn[C:2 * C][hs]
        bvs = b_attn[2 * C:3 * C][hs]
        bqk = np.stack(
            [bq[P * p:P * (p + 1)] for p in range(3)]
            + [bk[P * p:P * (p + 1)] for p in range(3)],
            axis=1,
        ).astype(np.float32)
        in_maps.append({
            "xT": np.ascontiguousarray(x[b].T).astype(NP_MM),
            "wq": np.ascontiguousarray(W_attn[:, 0:C][:, hs]).astype(NP_MM),
            "wk": np.ascontiguousarray(W_attn[:, C:2 * C][:, hs]).astype(NP_MM),
            "wv": np.ascontiguousarray(W_attn[:, 2 * C:3 * C][:, hs]).astype(NP_MM),
            "wp": np.ascontiguousarray(W_proj[hs, :]).astype(NP_MM),
            "bqk": np.ascontiguousarray(bqk),
            "bv": np.ascontiguousarray(bvs.reshape(1, GC)),
            "mask": mask,
        })
    return in_maps


def kernel(x, W_attn, b_attn, W_proj, b_proj, _trace=False):
    global last_results
    nc = _get_nc()
    in_maps = make_in_maps(x, W_attn, b_attn, W_proj)
    res = run_bass_kernel_spmd(nc, in_maps, list(range(NCORES)), trace=_trace)
    last_results = res
    out = np.zeros((B, T, C), np.float32)
    for core in range(NCORES):
        out[core // 2] += res.results[core]["out"]
    out += np.asarray(b_proj, np.float32)[None, None, :]
    return out


# revision 19
# speedup vs baseline: 1.0938x; 1.0938x over previous
"""Causal self-attention (B=4, T=2048, C=768, H=12) on 8 trn2 NeuronCores.

Sharding: core = (batch b in 0..3) x (head-group g in 0..1, 6 heads each).
Each core: QKV projection for its 6 heads, causal attention, partial output
projection (its heads' rows of W_proj). Host sums the two partials per batch
and adds b_proj.

Device-side layout (per core):
  xT [768, 2048]  (host pre-transposes x[b])
  qT/kT produced as [d, t] pair tiles (lhsT = W-slice, rhs = xT)
  v produced natural [t, d] with a ones column appended per head
  S^T [k, q] = kT_block.T @ qT  -> exp on ACT -> PV: y^T += v_aug.T @ expS
    row 64 of the PV accumulator = softmax denominator (ones-column trick)
  normalize via DVE reciprocal + gpsimd partition_broadcast + DVE multiply
  out_partial[t, :] = sum_h yT_h.T @ Wp_h   (y^T is directly the proj lhsT)

Matmul operands are stored bf16 (full PE rate, half the HBM traffic);
accumulation is fp32 in PSUM; the output partials are fp32.
"""

import sys

for _p in ("/opt/pypackages", "/opt/trn_rl_repo"):
    if _p not in sys.path:
        sys.path.insert(0, _p)

import numpy as np
import ml_dtypes

import concourse.bass as bass
import concourse.tile as tile
from concourse import bacc, mybir
from concourse.bass_utils import run_bass_kernel_spmd

B, T, C, H = 4, 2048, 768, 12
HS = C // H            # 64 head dim
HPC = 6                # heads per core
GC = HPC * HS          # 384 columns per core
NCORES = 8
NK = C // 128          # 6 contraction tiles over c_in
P = 128
F32 = mybir.dt.float32
MM = mybir.dt.bfloat16   # matmul operand dtype
NP_MM = ml_dtypes.bfloat16

NQCH = T // 512        # 4 q-chunks of 512
NTB = T // P           # 16 token blocks of 128


def _build_nc():
    nc = bacc.Bacc("TRN2")

    xT = nc.declare_dram_parameter("xT", [C, T], MM, isOutput=False)
    wq = nc.declare_dram_parameter("wq", [C, GC], MM, isOutput=False)
    wk = nc.declare_dram_parameter("wk", [C, GC], MM, isOutput=False)
    wv = nc.declare_dram_parameter("wv", [C, GC], MM, isOutput=False)
    wp = nc.declare_dram_parameter("wp", [GC, C], MM, isOutput=False)
    bqk = nc.declare_dram_parameter("bqk", [P, 6], F32, isOutput=False)
    bv = nc.declare_dram_parameter("bv", [1, GC], F32, isOutput=False)
    mask = nc.declare_dram_parameter("mask", [P, P], MM, isOutput=False)
    out = nc.declare_dram_parameter("out", [T, C], F32, isOutput=True)

    xTv = xT.ap().rearrange("(k p) t -> k p t", p=P)
    wqv = wq.ap().rearrange("(k p) d -> k p d", p=P)
    wkv = wk.ap().rearrange("(k p) d -> k p d", p=P)
    wvv = wv.ap().rearrange("(k p) d -> k p d", p=P)
    wpv = wp.ap().rearrange("(h p) n -> h p n", p=HS)
    outv = out.ap().rearrange("(b p) n -> b p n", p=P)

    with tile.TileContext(nc) as tc:
        from contextlib import ExitStack

        with ExitStack() as ctx:
            pers = ctx.enter_context(tc.tile_pool(name="pers", bufs=1))
            # PSUM: psMM 3 x [128,1024] (2 banks each) + psY 2 x 1 bank = 8 banks
            psMM = ctx.enter_context(tc.tile_pool(name="psMM", bufs=3, space="PSUM"))
            psY = ctx.enter_context(tc.tile_pool(name="psY", bufs=2, space="PSUM"))

            # ---- persistent tiles ----
            qkT = [pers.tile([P, T], MM, name=f"qkT{i}") for i in range(6)]
            vsb = [pers.tile([P, HPC * (HS + 1)], MM, name=f"v{tb}") for tb in range(NTB)]
            wph = [pers.tile([HS, C], MM, name=f"wp{h}") for h in range(HPC)]
            mask_sb = pers.tile([P, P], MM, name="mask")
            bqk_sb = pers.tile([P, 6], F32, name="bqk")
            bv_sb = pers.tile([1, GC], F32, name="bv")
            bvb = pers.tile([P, GC], F32, name="bvb")
            # ones rows for PE-based partition broadcasts (K=1 matmuls)
            ones128 = pers.tile([1, P], MM, name="ones128")
            ones65 = pers.tile([HS + 1, HS], MM, name="ones65")
            bv_bf = pers.tile([1, GC], MM, name="bv_bf")

            nc.sync.dma_start(mask_sb[:], mask.ap())
            nc.sync.dma_start(bqk_sb[:], bqk.ap())
            nc.sync.dma_start(bv_sb[:], bv.ap())
            for h in range(HPC):
                nc.sync.dma_start(wph[h][:], wpv[h])
            nc.vector.memset(ones128[:], 1.0)
            nc.vector.memset(ones65[:], 1.0)
            # bvb[p, :] = bv for all p  (PE broadcast through PSUM); stage bv
            # through a DVE copy so the matmul's producers share one sem
            with nc.allow_low_precision(reason="bias broadcast staging in bf16"):
                nc.vector.tensor_copy(out=bv_bf[:], in_=bv_sb[:])
            ps_b = psMM.tile([P, 1024], F32, tag="mm", name="ps_b")
            nc.tensor.matmul(ps_b[:, 0:GC], ones128[:], bv_bf[:],
                             start=True, stop=True)
            nc.vector.tensor_copy(out=bvb[:], in_=ps_b[:, 0:GC])

            # ---- QKV (xT and W tiles live only for this phase) ----
            with tc.tile_pool(name="wx", bufs=1) as wx:
                xt = [wx.tile([P, T], MM, name=f"xt{k}") for k in range(NK)]
                wqt = [wx.tile([P, GC], MM, name=f"wq{k}") for k in range(NK)]
                wkt = [wx.tile([P, GC], MM, name=f"wk{k}") for k in range(NK)]
                wvt = [wx.tile([P, GC], MM, name=f"wv{k}") for k in range(NK)]
                for k in range(NK):
                    nc.sync.dma_start(xt[k][:], xTv[k])
                    nc.sync.dma_start(wqt[k][:], wqv[k])
                    nc.sync.dma_start(wkt[k][:], wkv[k])
                    nc.sync.dma_start(wvt[k][:], wvv[k])

                # qT / kT pair tiles: i in 0..2 -> q pair i; 3..5 -> k pair i-3
                for i in range(6):
                    wt = wqt if i < 3 else wkt
                    p = i % 3
                    for nch in range(NQCH):
                        ps = psMM.tile([P, 1024], F32, tag="mm", name="ps_qk")
                        for k in range(NK):
                            nc.tensor.matmul(
                                ps[:, 0:512],
                                wt[k][:, P * p:P * (p + 1)],
                                xt[k][:, 512 * nch:512 * (nch + 1)],
                                start=(k == 0),
                                stop=(k == NK - 1),
                            )
                        nc.vector.tensor_scalar_add(
                            out=qkT[i][:, 512 * nch:512 * (nch + 1)],
                            in0=ps[:, 0:512],
                            scalar1=bqk_sb[:, i:i + 1],
                        )

                # v natural [t, d] + bias, plus ones column per head
                for tb in range(NTB):
                    ps = psMM.tile([P, 1024], F32, tag="mm", name="ps_v")
                    for k in range(NK):
                        nc.tensor.matmul(
                            ps[:, 0:GC],
                            xt[k][:, P * tb:P * (tb + 1)],
                            wvt[k][:],
                            start=(k == 0),
                            stop=(k == NK - 1),
                        )
                    v3 = vsb[tb].rearrange("p (h e) -> p h e", e=HS + 1)
                    nc.vector.tensor_add(
                        out=v3[:, :, 0:HS],
                        in0=ps[:, 0:GC].rearrange("p (h d) -> p h d", d=HS),
                        in1=bvb.rearrange("p (h d) -> p h d", d=HS),
                    )
                    nc.vector.memset(v3[:, :, HS:HS + 1], 1.0)

            # ---- attention + projection, chunk-outer ----
            work = ctx.enter_context(tc.tile_pool(name="work", bufs=3))
            ytp = ctx.enter_context(tc.tile_pool(name="ytp", bufs=2))

            def emit_norm(yps, h):
                # 1/sum via exp(-ln(sum)) on ScalarE (fast, reads PSUM, and
                # keeps the slow DVE reciprocal off the PE critical path)
                lnr = work.tile([HS + 1, 512], F32, tag="lnr", name="lnr")
                nc.scalar.activation(
                    out=lnr[HS:HS + 1, :], in_=yps[HS:HS + 1, :],
                    func=mybir.ActivationFunctionType.Ln)
                rt = work.tile([HS + 1, 512], MM, tag="rt", name="rt")
                nc.scalar.activation(
                    out=rt[HS:HS + 1, :], in_=lnr[HS:HS + 1, :],
                    func=mybir.ActivationFunctionType.Exp, scale=-1.0)
                # broadcast across the 64 head-dim partitions via K=1 matmul
                rb = psMM.tile([P, 1024], F32, tag="mm", name="rb")
                nc.tensor.matmul(rb[0:HS, 0:512], ones65[HS:HS + 1, :],
                                 rt[HS:HS + 1, :], start=True, stop=True)
                rb_sb = work.tile([HS, 512], F32, tag="rb", name="rb_sb")
                nc.vector.tensor_copy(out=rb_sb[:], in_=rb[0:HS, 0:512])
                yt = ytp.tile([HS, 512], MM, tag=f"yt{h}", name=f"yt{h}")
                nc.vector.tensor_mul(out=yt[:], in0=yps[0:HS, :], in1=rb_sb[:])
                return yt

            for c in range(NQCH):
                ytiles = [None] * HPC
                pend = None
                for h in range(HPC):
                    hp, hd = divmod(h, 2)
                    qTh = qkT[hp][HS * hd:HS * (hd + 1), :]
                    kTh = qkT[3 + hp][HS * hd:HS * (hd + 1), :]
                    vh = [
                        vsb[j].rearrange("p (h e) -> p h e", e=HS + 1)[:, h, :]
                        for j in range(4 * c + 4)
                    ]

                    yps = psY.tile([HS + 1, 512], F32, tag="y", name="yps")
                    jlast = 4 * c + 3
                    for jp in range(2 * c + 2):
                        sps = psMM.tile([P, 1024], F32, tag="mm", name="ps_s")
                        es = work.tile([P, 1024], MM, tag="es", name="es")
                        for u in range(2):
                            j = 2 * jp + u
                            m = j - 4 * c
                            qs = P * m if m > 0 else 0
                            nc.tensor.matmul(
                                sps[:, 512 * u + qs:512 * (u + 1)],
                                kTh[:, P * j:P * (j + 1)],
                                qTh[:, 512 * c + qs:512 * (c + 1)],
                                start=True,
                                stop=True,
                            )
                        if 2 * jp + 1 - 4 * c > 0:
                            # pair contains diagonal blocks: exp only the
                            # initialized subrange of each half
                            for u in range(2):
                                j = 2 * jp + u
                                m = j - 4 * c
                                qs = P * m if m > 0 else 0
                                nc.scalar.activation(
                                    out=es[:, 512 * u + qs:512 * (u + 1)],
                                    in_=sps[:, 512 * u + qs:512 * (u + 1)],
                                    func=mybir.ActivationFunctionType.Exp,
                                    scale=1.0 / 8.0,
                                )
                        else:
                            nc.scalar.activation(
                                out=es[:],
                                in_=sps[:],
                                func=mybir.ActivationFunctionType.Exp,
                                scale=1.0 / 8.0,
                            )
                        for u in range(2):
                            j = 2 * jp + u
                            m = j - 4 * c
                            qs = P * m if m > 0 else 0
                            if m >= 0:
                                # mask the 128x128 diagonal sub-block
                                nc.vector.tensor_mul(
                                    out=es[:, 512 * u + qs:512 * u + qs + P],
                                    in0=es[:, 512 * u + qs:512 * u + qs + P],
                                    in1=mask_sb[:],
                                )
                            nc.tensor.matmul(
                                yps[:, qs:512],
                                vh[j],
                                es[:, 512 * u + qs:512 * (u + 1)],
                                start=(j == 0),
                                stop=(j == jlast),
                            )

                    # normalization for the PREVIOUS head is emitted here so
                    # the PE queue never waits on the denominator chain
                    if pend is not None:
                        ph, pyps = pend
                        ytiles[ph] = emit_norm(pyps, ph)
                    pend = (h, yps)

                ph, pyps = pend
                ytiles[ph] = emit_norm(pyps, ph)

                # projection for the 4 token-blocks of this chunk
                for tq in range(4):
                    tb = 4 * c + tq
                    pps = psMM.tile([P, 1024], F32, tag="mm", name="ps_o")
                    for h in range(HPC):
                        for n0, nn in ((0, 512), (512, 256)):
                            nc.tensor.matmul(
                                pps[:, n0:n0 + nn],
                                ytiles[h][:, P * tq:P * (tq + 1)],
                                wph[h][:, n0:n0 + nn],
                                start=(h == 0),
                                stop=(h == HPC - 1),
                            )
                    ot = work.tile([P, C], F32, tag="ot", name="ot")
                    nc.vector.tensor_copy(out=ot[:], in_=pps[:, 0:C])
                    nc.sync.dma_start(outv[tb], ot[:])

    nc.compile()
    return nc


_nc_cache = None
last_results = None


def _get_nc():
    global _nc_cache
    if _nc_cache is None:
        _nc_cache = _build_nc()
    return _nc_cache


def make_in_maps(x, W_attn, b_attn, W_proj):
    x = np.asarray(x, np.float32)
    W_attn = np.asarray(W_attn, np.float32)
    b_attn = np.asarray(b_attn, np.float32)
    W_proj = np.asarray(W_proj, np.float32)

    kk, qq = np.meshgrid(np.arange(P), np.arange(P), indexing="ij")
    mask = (qq >= kk).astype(NP_MM)

    in_maps = []
    for core in range(NCORES):
        b, g = divmod(core, 2)
        hs = slice(GC * g, GC * (g + 1))
        bq = b_attn[0:C][hs]
        bk = b_attn[C:2 * C][hs]
        bvs = b_attn[2 * C:3 * C][hs]
        bqk = np.stack(
            [bq[P * p:P * (p + 1)] for p in range(3)]
            + [bk[P * p:P * (p + 1)] for p in range(3)],
            axis=1,
        ).astype(np.float32)
        in_maps.append({
            "xT": np.ascontiguousarray(x[b].T).astype(NP_MM),
            "wq": np.ascontiguousarray(W_attn[:, 0:C][:, hs]).astype(NP_MM),
            "wk": np.ascontiguousarray(W_attn[:, C:2 * C][:, hs]).astype(NP_MM),
            "wv": np.ascontiguousarray(W_attn[:, 2 * C:3 * C][:, hs]).astype(NP_MM),
            "wp": np.ascontiguousarray(W_proj[hs, :]).astype(NP_MM),
            "bqk": np.ascontiguousarray(bqk),
            "bv": np.ascontiguousarray(bvs.reshape(1, GC)),
            "mask": mask,
        })
    return in_maps


def kernel(x, W_attn, b_attn, W_proj, b_proj, _trace=False):
    global last_results
    nc = _get_nc()
    in_maps = make_in_maps(x, W_attn, b_attn, W_proj)
    res = run_bass_kernel_spmd(nc, in_maps, list(range(NCORES)), trace=_trace)
    last_results = res
    out = np.zeros((B, T, C), np.float32)
    for core in range(NCORES):
        out[core // 2] += res.results[core]["out"]
    out += np.asarray(b_proj, np.float32)[None, None, :]
    return out


# revision 21
# speedup vs baseline: 1.2195x; 1.1149x over previous
"""Causal self-attention (B=4, T=2048, C=768, H=12) on 8 trn2 NeuronCores.

Sharding: core = (batch b in 0..3) x (head-group g in 0..1, 6 heads each).
Each core: QKV projection for its 6 heads, causal attention, partial output
projection (its heads' rows of W_proj). Host sums the two partials per batch
and adds b_proj.

Device-side layout (per core):
  xT [768, 2048]  (host pre-transposes x[b])
  qT/kT produced as [d, t] pair tiles (lhsT = W-slice, rhs = xT)
  v produced natural [t, d] with a ones column appended per head
  S^T [k, q] = kT_block.T @ qT  -> exp on ACT -> PV: y^T += v_aug.T @ expS
    row 64 of the PV accumulator = softmax denominator (ones-column trick)
  normalize via DVE reciprocal + gpsimd partition_broadcast + DVE multiply
  out_partial[t, :] = sum_h yT_h.T @ Wp_h   (y^T is directly the proj lhsT)

Matmul operands are stored bf16 (full PE rate, half the HBM traffic);
accumulation is fp32 in PSUM; the output partials are fp32.
"""

import sys

for _p in ("/opt/pypackages", "/opt/trn_rl_repo"):
    if _p not in sys.path:
        sys.path.insert(0, _p)

import numpy as np
import ml_dtypes

import concourse.bass as bass
import concourse.tile as tile
from concourse import bacc, mybir
from concourse.bass_utils import run_bass_kernel_spmd

B, T, C, H = 4, 2048, 768, 12
HS = C // H            # 64 head dim
HPC = 6                # heads per core
GC = HPC * HS          # 384 columns per core
NCORES = 8
NK = C // 128          # 6 contraction tiles over c_in
P = 128
F32 = mybir.dt.float32
MM = mybir.dt.bfloat16   # matmul operand dtype
NP_MM = ml_dtypes.bfloat16

NQCH = T // 512        # 4 q-chunks of 512
NTB = T // P           # 16 token blocks of 128


def _build_nc():
    nc = bacc.Bacc("TRN2")

    xT = nc.declare_dram_parameter("xT", [C, T], MM, isOutput=False)
    wq = nc.declare_dram_parameter("wq", [C, GC], MM, isOutput=False)
    wk = nc.declare_dram_parameter("wk", [C, GC], MM, isOutput=False)
    wv = nc.declare_dram_parameter("wv", [C, GC], MM, isOutput=False)
    wp = nc.declare_dram_parameter("wp", [GC, C], MM, isOutput=False)
    bqk = nc.declare_dram_parameter("bqk", [P, 6], F32, isOutput=False)
    bv = nc.declare_dram_parameter("bv", [1, GC], F32, isOutput=False)
    mask = nc.declare_dram_parameter("mask", [P, P], MM, isOutput=False)
    out = nc.declare_dram_parameter("out", [T, C], F32, isOutput=True)

    xTv = xT.ap().rearrange("(k p) t -> k p t", p=P)
    wqv = wq.ap().rearrange("(k p) d -> k p d", p=P)
    wkv = wk.ap().rearrange("(k p) d -> k p d", p=P)
    wvv = wv.ap().rearrange("(k p) d -> k p d", p=P)
    wpv = wp.ap().rearrange("(h p) n -> h p n", p=HS)
    outv = out.ap().rearrange("(b p) n -> b p n", p=P)

    with tile.TileContext(nc) as tc:
        from contextlib import ExitStack

        with ExitStack() as ctx:
            pers = ctx.enter_context(tc.tile_pool(name="pers", bufs=1))
            # PSUM: psMM 3 x [128,1024] (2 banks each) + psY 2 x 1 bank = 8 banks
            psMM = ctx.enter_context(tc.tile_pool(name="psMM", bufs=3, space="PSUM"))
            psY = ctx.enter_context(tc.tile_pool(name="psY", bufs=2, space="PSUM"))

            # ---- persistent tiles ----
            qkT = [pers.tile([P, T], MM, name=f"qkT{i}") for i in range(6)]
            vsb = [pers.tile([P, HPC * (HS + 1)], MM, name=f"v{tb}") for tb in range(NTB)]
            wph = [pers.tile([HS, C], MM, name=f"wp{h}") for h in range(HPC)]
            mask_sb = pers.tile([P, P], MM, name="mask")
            bqk_sb = pers.tile([P, 6], F32, name="bqk")
            bv_sb = pers.tile([1, GC], F32, name="bv")
            bvb = pers.tile([P, GC], F32, name="bvb")
            # ones rows for PE-based partition broadcasts (K=1 matmuls)
            ones128 = pers.tile([1, P], MM, name="ones128")
            ones65 = pers.tile([HS + 1, HS], MM, name="ones65")
            bv_bf = pers.tile([1, GC], MM, name="bv_bf")

            nc.sync.dma_start(mask_sb[:], mask.ap())
            nc.sync.dma_start(bqk_sb[:], bqk.ap())
            nc.sync.dma_start(bv_sb[:], bv.ap())
            for h in range(HPC):
                nc.sync.dma_start(wph[h][:], wpv[h])
            nc.vector.memset(ones128[:], 1.0)
            nc.vector.memset(ones65[:], 1.0)
            # bvb[p, :] = bv for all p  (PE broadcast through PSUM); stage bv
            # through a DVE copy so the matmul's producers share one sem
            with nc.allow_low_precision(reason="bias broadcast staging in bf16"):
                nc.vector.tensor_copy(out=bv_bf[:], in_=bv_sb[:])
            ps_b = psMM.tile([P, 1024], F32, tag="mm", name="ps_b")
            nc.tensor.matmul(ps_b[:, 0:GC], ones128[:], bv_bf[:],
                             start=True, stop=True)
            nc.vector.tensor_copy(out=bvb[:], in_=ps_b[:, 0:GC])

            # ---- QKV (xT and W tiles live only for this phase) ----
            with tc.tile_pool(name="wx", bufs=1) as wx:
                xt = [wx.tile([P, T], MM, name=f"xt{k}") for k in range(NK)]
                wqt = [wx.tile([P, GC], MM, name=f"wq{k}") for k in range(NK)]
                wkt = [wx.tile([P, GC], MM, name=f"wk{k}") for k in range(NK)]
                wvt = [wx.tile([P, GC], MM, name=f"wv{k}") for k in range(NK)]
                # xt+wq first: the first QKV accumulation group consumes them
                for k in range(NK):
                    nc.sync.dma_start(xt[k][:], xTv[k])
                    nc.sync.dma_start(wqt[k][:], wqv[k])
                for k in range(NK):
                    nc.sync.dma_start(wkt[k][:], wkv[k])
                    nc.sync.dma_start(wvt[k][:], wvv[k])

                # qT / kT pair tiles: i in 0..2 -> q pair i; 3..5 -> k pair i-3
                for i in range(6):
                    wt = wqt if i < 3 else wkt
                    p = i % 3
                    for nch in range(NQCH):
                        ps = psMM.tile([P, 1024], F32, tag="mm", name="ps_qk")
                        for k in range(NK):
                            nc.tensor.matmul(
                                ps[:, 0:512],
                                wt[k][:, P * p:P * (p + 1)],
                                xt[k][:, 512 * nch:512 * (nch + 1)],
                                start=(k == 0),
                                stop=(k == NK - 1),
                            )
                        nc.vector.tensor_scalar_add(
                            out=qkT[i][:, 512 * nch:512 * (nch + 1)],
                            in0=ps[:, 0:512],
                            scalar1=bqk_sb[:, i:i + 1],
                        )

                # v natural [t, d] + bias, plus ones column per head
                for tb in range(NTB):
                    ps = psMM.tile([P, 1024], F32, tag="mm", name="ps_v")
                    for k in range(NK):
                        nc.tensor.matmul(
                            ps[:, 0:GC],
                            xt[k][:, P * tb:P * (tb + 1)],
                            wvt[k][:],
                            start=(k == 0),
                            stop=(k == NK - 1),
                        )
                    v3 = vsb[tb].rearrange("p (h e) -> p h e", e=HS + 1)
                    nc.vector.tensor_add(
                        out=v3[:, :, 0:HS],
                        in0=ps[:, 0:GC].rearrange("p (h d) -> p h d", d=HS),
                        in1=bvb.rearrange("p (h d) -> p h d", d=HS),
                    )
                    nc.vector.memset(v3[:, :, HS:HS + 1], 1.0)

            # ---- attention + projection, chunk-outer ----
            work = ctx.enter_context(tc.tile_pool(name="work", bufs=3))
            ytp = ctx.enter_context(tc.tile_pool(name="ytp", bufs=2))

            def emit_norm(yps, h):
                # copy the sums row to SBUF (DVE), broadcast the RAW sums
                # across the 64 head-dim partitions via a K=1 PE matmul, then
                # one fast approximate reciprocal (custom DVE, ~51 ULP) on the
                # broadcast, and multiply. No transcendental tables, no ACT.
                st = work.tile([HS + 1, 512], MM, tag="st", name="st")
                with nc.allow_low_precision(reason="softmax denom staged bf16"):
                    nc.vector.tensor_copy(out=st[HS:HS + 1, :],
                                          in_=yps[HS:HS + 1, :])
                rb = psMM.tile([P, 1024], F32, tag="mm", name="rb")
                nc.tensor.matmul(rb[0:HS, 0:512], ones65[HS:HS + 1, :],
                                 st[HS:HS + 1, :], start=True, stop=True)
                rbi = work.tile([HS, 512], F32, tag="rbi", name="rbi")
                nc.vector.reciprocal_approx_fast(out=rbi[:], in_=rb[0:HS, 0:512])
                yt = ytp.tile([HS, 512], MM, tag=f"yt{h}", name=f"yt{h}")
                nc.vector.tensor_mul(out=yt[:], in0=yps[0:HS, :], in1=rbi[:])
                return yt

            for c in range(NQCH):
                ytiles = [None] * HPC
                pend = None
                for h in range(HPC):
                    hp, hd = divmod(h, 2)
                    qTh = qkT[hp][HS * hd:HS * (hd + 1), :]
                    kTh = qkT[3 + hp][HS * hd:HS * (hd + 1), :]
                    vh = [
                        vsb[j].rearrange("p (h e) -> p h e", e=HS + 1)[:, h, :]
                        for j in range(4 * c + 4)
                    ]

                    yps = psY.tile([HS + 1, 512], F32, tag="y", name="yps")
                    jlast = 4 * c + 3
                    for jp in range(2 * c + 2):
                        sps = psMM.tile([P, 1024], F32, tag="mm", name="ps_s")
                        es = work.tile([P, 1024], MM, tag="es", name="es")
                        for u in range(2):
                            j = 2 * jp + u
                            m = j - 4 * c
                            qs = P * m if m > 0 else 0
                            nc.tensor.matmul(
                                sps[:, 512 * u + qs:512 * (u + 1)],
                                kTh[:, P * j:P * (j + 1)],
                                qTh[:, 512 * c + qs:512 * (c + 1)],
                                start=True,
                                stop=True,
                            )
                        if 2 * jp + 1 - 4 * c > 0:
                            # pair contains diagonal blocks: exp only the
                            # initialized subrange of each half
                            for u in range(2):
                                j = 2 * jp + u
                                m = j - 4 * c
                                qs = P * m if m > 0 else 0
                                nc.scalar.activation(
                                    out=es[:, 512 * u + qs:512 * (u + 1)],
                                    in_=sps[:, 512 * u + qs:512 * (u + 1)],
                                    func=mybir.ActivationFunctionType.Exp,
                                    scale=1.0 / 8.0,
                                )
                        else:
                            nc.scalar.activation(
                                out=es[:],
                                in_=sps[:],
                                func=mybir.ActivationFunctionType.Exp,
                                scale=1.0 / 8.0,
                            )
                        for u in range(2):
                            j = 2 * jp + u
                            m = j - 4 * c
                            qs = P * m if m > 0 else 0
                            if m >= 0:
                                # mask the 128x128 diagonal sub-block
                                nc.vector.tensor_mul(
                                    out=es[:, 512 * u + qs:512 * u + qs + P],
                                    in0=es[:, 512 * u + qs:512 * u + qs + P],
                                    in1=mask_sb[:],
                                )
                            nc.tensor.matmul(
                                yps[:, qs:512],
                                vh[j],
                                es[:, 512 * u + qs:512 * (u + 1)],
                                start=(j == 0),
                                stop=(j == jlast),
                            )

                    # normalization for the PREVIOUS head is emitted here so
                    # the PE queue never waits on the denominator chain
                    if pend is not None:
                        ph, pyps = pend
                        ytiles[ph] = emit_norm(pyps, ph)
                    pend = (h, yps)

                ph, pyps = pend
                ytiles[ph] = emit_norm(pyps, ph)

                # projection for the 4 token-blocks of this chunk
                for tq in range(4):
                    tb = 4 * c + tq
                    pps = psMM.tile([P, 1024], F32, tag="mm", name="ps_o")
                    for h in range(HPC):
                        for n0, nn in ((0, 512), (512, 256)):
                            nc.tensor.matmul(
                                pps[:, n0:n0 + nn],
                                ytiles[h][:, P * tq:P * (tq + 1)],
                                wph[h][:, n0:n0 + nn],
                                start=(h == 0),
                                stop=(h == HPC - 1),
                            )
                    ot = work.tile([P, C], F32, tag="ot", name="ot")
                    nc.vector.tensor_copy(out=ot[:], in_=pps[:, 0:C])
                    nc.sync.dma_start(outv[tb], ot[:])

    nc.compile()
    return nc


_nc_cache = None
last_results = None


def _get_nc():
    global _nc_cache
    if _nc_cache is None:
        _nc_cache = _build_nc()
    return _nc_cache


def make_in_maps(x, W_attn, b_attn, W_proj):
    x = np.asarray(x, np.float32)
    W_attn = np.asarray(W_attn, np.float32)
    b_attn = np.asarray(b_attn, np.float32)
    W_proj = np.asarray(W_proj, np.float32)

    kk, qq = np.meshgrid(np.arange(P), np.arange(P), indexing="ij")
    mask = (qq >= kk).astype(NP_MM)

    in_maps = []
    for core in range(NCORES):
        b, g = divmod(core, 2)
        hs = slice(GC * g, GC * (g + 1))
        bq = b_attn[0:C][hs]
        bk = b_attn[C:2 * C][hs]
        bvs = b_attn[2 * C:3 * C][hs]
        bqk = np.stack(
            [bq[P * p:P * (p + 1)] for p in range(3)]
            + [bk[P * p:P * (p + 1)] for p in range(3)],
            axis=1,
        ).astype(np.float32)
        in_maps.append({
            "xT": np.ascontiguousarray(x[b].T).astype(NP_MM),
            "wq": np.ascontiguousarray(W_attn[:, 0:C][:, hs]).astype(NP_MM),
            "wk": np.ascontiguousarray(W_attn[:, C:2 * C][:, hs]).astype(NP_MM),
            "wv": np.ascontiguousarray(W_attn[:, 2 * C:3 * C][:, hs]).astype(NP_MM),
            "wp": np.ascontiguousarray(W_proj[hs, :]).astype(NP_MM),
            "bqk": np.ascontiguousarray(bqk),
            "bv": np.ascontiguousarray(bvs.reshape(1, GC)),
            "mask": mask,
        })
    return in_maps


def kernel(x, W_attn, b_attn, W_proj, b_proj, _trace=False):
    global last_results
    nc = _get_nc()
    in_maps = make_in_maps(x, W_attn, b_attn, W_proj)
    res = run_bass_kernel_spmd(nc, in_maps, list(range(NCORES)), trace=_trace)
    last_results = res
    out = np.zeros((B, T, C), np.float32)
    for core in range(NCORES):
        out[core // 2] += res.results[core]["out"]
    out += np.asarray(b_proj, np.float32)[None, None, :]
    return out


# revision 27
# speedup vs baseline: 1.3999x; 1.1479x over previous
"""Causal self-attention (B=4, T=2048, C=768, H=12) on 8 trn2 NeuronCores.

Sharding: core = (batch b in 0..3) x (head-group g in 0..1, 6 heads each).
Each core: QKV projection for its 6 heads, causal attention, partial output
projection (its heads' rows of W_proj). Host sums the two partials per batch
and adds b_proj.

Device-side layout (per core):
  xT [768, 2048]  (host pre-transposes x[b])
  qT/kT produced as [d, t] pair tiles (lhsT = W-slice, rhs = xT)
  v produced natural [t, d] with a ones column appended per head
  S^T [k, q] = kT_block.T @ qT  -> exp on ACT -> PV: y^T += v_aug.T @ expS
    row 64 of the PV accumulator = softmax denominator (ones-column trick)
  normalize via DVE reciprocal + gpsimd partition_broadcast + DVE multiply
  out_partial[t, :] = sum_h yT_h.T @ Wp_h   (y^T is directly the proj lhsT)

Matmul operands are stored bf16 (full PE rate, half the HBM traffic);
accumulation is fp32 in PSUM; the output partials are fp32.
"""

import sys

for _p in ("/opt/pypackages", "/opt/trn_rl_repo"):
    if _p not in sys.path:
        sys.path.insert(0, _p)

import numpy as np
import ml_dtypes

import concourse.bass as bass
import concourse.tile as tile
from concourse import bacc, mybir
from concourse.bass_utils import run_bass_kernel_spmd

B, T, C, H = 4, 2048, 768, 12
HS = C // H            # 64 head dim
HPC = 6                # heads per core
GC = HPC * HS          # 384 columns per core
NCORES = 8
NK = C // 128          # 6 contraction tiles over c_in
P = 128
F32 = mybir.dt.float32
MM = mybir.dt.bfloat16   # matmul operand dtype
NP_MM = ml_dtypes.bfloat16

NQCH = T // 512        # 4 q-chunks of 512
NTB = T // P           # 16 token blocks of 128


def _build_nc():
    nc = bacc.Bacc("TRN2")

    xT = nc.declare_dram_parameter("xT", [C, T], MM, isOutput=False)
    wq = nc.declare_dram_parameter("wq", [C, GC], MM, isOutput=False)
    wk = nc.declare_dram_parameter("wk", [C, GC], MM, isOutput=False)
    wv = nc.declare_dram_parameter("wv", [C, GC], MM, isOutput=False)
    wp = nc.declare_dram_parameter("wp", [GC, C], MM, isOutput=False)
    bqk = nc.declare_dram_parameter("bqk", [P, 6], F32, isOutput=False)
    bv = nc.declare_dram_parameter("bv", [1, GC], F32, isOutput=False)
    mask = nc.declare_dram_parameter("mask", [P, P], MM, isOutput=False)
    out = nc.declare_dram_parameter("out", [T, C], F32, isOutput=True)

    xTv = xT.ap().rearrange("(k p) t -> k p t", p=P)
    wqv = wq.ap().rearrange("(k p) d -> k p d", p=P)
    wkv = wk.ap().rearrange("(k p) d -> k p d", p=P)
    wvv = wv.ap().rearrange("(k p) d -> k p d", p=P)
    wpv = wp.ap().rearrange("(h p) n -> h p n", p=P)
    outv = out.ap().rearrange("(b p) n -> b p n", p=P)

    with tile.TileContext(nc) as tc:
        from contextlib import ExitStack

        with ExitStack() as ctx:
            pers = ctx.enter_context(tc.tile_pool(name="pers", bufs=1))
            # PSUM: psMM 2 x [128,1024] (2 banks each) + psY 4 x 1 bank = 8 banks
            psMM = ctx.enter_context(tc.tile_pool(name="psMM", bufs=2, space="PSUM"))
            psY = ctx.enter_context(tc.tile_pool(name="psY", bufs=4, space="PSUM"))

            # ---- persistent tiles ----
            # v layout per head-PAIR block of 192 cols: [v_even(64) | ones(1) |
            # zeros(63) | v_odd(64)].  lhsT_even = cols[0:65] -> y at rows 0-63,
            # sums at row 64; lhsT_odd = cols[64:192] -> sums at row 0, y at
            # rows 64-127.  Odd heads' y lands on partitions 64-127 so the
            # pair's projection lhsT is a single [128, t] tile (K=128 matmuls).
            VPB = 3 * HS  # 192 cols per pair block
            qkT = [pers.tile([P, T], MM, name=f"qkT{i}") for i in range(6)]
            vsb = [pers.tile([P, 3 * VPB], MM, name=f"v{tb}") for tb in range(NTB)]
            wph = [pers.tile([P, C], MM, name=f"wp{hp}") for hp in range(3)]
            mask_sb = pers.tile([P, P], MM, name="mask")
            bqk_sb = pers.tile([P, 6], F32, name="bqk")
            bv_sb = pers.tile([1, GC], F32, name="bv")
            bvb = pers.tile([P, GC], F32, name="bvb")
            # ones rows for PE-based partition broadcasts (K=1 matmuls)
            ones128 = pers.tile([1, P], MM, name="ones128")
            ones65 = pers.tile([HS + 1, HS], MM, name="ones65")
            bv_bf = pers.tile([1, GC], MM, name="bv_bf")

            nc.sync.dma_start(mask_sb[:], mask.ap())
            nc.sync.dma_start(bqk_sb[:], bqk.ap())
            nc.sync.dma_start(bv_sb[:], bv.ap())
            for hp in range(3):
                nc.sync.dma_start(wph[hp][:], wpv[hp])
            nc.vector.memset(ones128[:], 1.0)
            nc.vector.memset(ones65[:], 1.0)
            # bvb[p, :] = bv for all p  (PE broadcast through PSUM); stage bv
            # through a DVE copy so the matmul's producers share one sem
            with nc.allow_low_precision(reason="bias broadcast staging in bf16"):
                nc.vector.tensor_copy(out=bv_bf[:], in_=bv_sb[:])
            ps_b = psMM.tile([P, 1024], F32, tag="mm", name="ps_b")
            nc.tensor.matmul(ps_b[:, 0:GC], ones128[:], bv_bf[:],
                             start=True, stop=True)
            nc.vector.tensor_copy(out=bvb[:], in_=ps_b[:, 0:GC])

            # ---- QKV (xT and W tiles live only for this phase) ----
            with tc.tile_pool(name="wx", bufs=1) as wx:
                xt = [wx.tile([P, T], MM, name=f"xt{k}") for k in range(NK)]
                wqt = [wx.tile([P, GC], MM, name=f"wq{k}") for k in range(NK)]
                wkt = [wx.tile([P, GC], MM, name=f"wk{k}") for k in range(NK)]
                wvt = [wx.tile([P, GC], MM, name=f"wv{k}") for k in range(NK)]
                # xt+wq first: the first QKV accumulation group consumes them
                for k in range(NK):
                    nc.sync.dma_start(xt[k][:], xTv[k])
                    nc.sync.dma_start(wqt[k][:], wqv[k])
                for k in range(NK):
                    nc.sync.dma_start(wkt[k][:], wkv[k])
                    nc.sync.dma_start(wvt[k][:], wvv[k])

                # qT / kT pair tiles: i in 0..2 -> q pair i; 3..5 -> k pair i-3
                for i in range(6):
                    wt = wqt if i < 3 else wkt
                    p = i % 3
                    for nch in range(NQCH):
                        ps = psMM.tile([P, 1024], F32, tag="mm", name="ps_qk")
                        for k in range(NK):
                            nc.tensor.matmul(
                                ps[:, 0:512],
                                wt[k][:, P * p:P * (p + 1)],
                                xt[k][:, 512 * nch:512 * (nch + 1)],
                                start=(k == 0),
                                stop=(k == NK - 1),
                            )
                        nc.vector.tensor_scalar_add(
                            out=qkT[i][:, 512 * nch:512 * (nch + 1)],
                            in0=ps[:, 0:512],
                            scalar1=bqk_sb[:, i:i + 1],
                        )

                # v natural [t, d] + bias, packed into pair blocks
                for tb in range(NTB):
                    ps = psMM.tile([P, 1024], F32, tag="mm", name="ps_v")
                    for k in range(NK):
                        nc.tensor.matmul(
                            ps[:, 0:GC],
                            xt[k][:, P * tb:P * (tb + 1)],
                            wvt[k][:],
                            start=(k == 0),
                            stop=(k == NK - 1),
                        )
                    v3 = vsb[tb].rearrange("p (b e) -> p b e", e=VPB)
                    ps4 = ps[:, 0:GC].rearrange("p (b o d) -> p b o d", o=2, d=HS)
                    bv4 = bvb.rearrange("p (b o d) -> p b o d", o=2, d=HS)
                    nc.vector.tensor_add(
                        out=v3[:, :, 0:HS],
                        in0=ps4[:, :, 0, :], in1=bv4[:, :, 0, :],
                    )
                    nc.vector.tensor_add(
                        out=v3[:, :, 2 * HS:3 * HS],
                        in0=ps4[:, :, 1, :], in1=bv4[:, :, 1, :],
                    )
                    nc.vector.memset(v3[:, :, HS:HS + 1], 1.0)
                    nc.vector.memset(v3[:, :, HS + 1:2 * HS], 0.0)

            # ---- attention + projection, chunk-outer ----
            work = ctx.enter_context(tc.tile_pool(name="work", bufs=3))
            ytp = ctx.enter_context(tc.tile_pool(name="ytp", bufs=2))

            def emit_norm_pair(hp, ypsA, ypsB):
                # Even head: sums at ypsA row 64 -> broadcast raw sums to
                # partitions 0-63 (K=1 PE matmul), reciprocal_approx_fast at
                # base 0 (custom DVE ops are broken at partition base != 0 on
                # HW), multiply.  Odd head: reciprocal FIRST at base 0 on the
                # [1,512] sums row, then broadcast the RECIPROCALS to
                # partitions 64-127 via the col-tiled K=1 matmul, evict with a
                # plain copy, multiply lane-aligned at base 64.
                st = work.tile([HS + 1, 512], MM, tag="st", name="st")
                with nc.allow_low_precision(reason="softmax denom staged bf16"):
                    nc.vector.tensor_copy(out=st[HS:HS + 1, :],
                                          in_=ypsA[HS:HS + 1, :])
                rb = psMM.tile([P, 1024], F32, tag="mm", name="rb")
                nc.tensor.matmul(rb[0:HS, 0:512], ones65[HS:HS + 1, :],
                                 st[HS:HS + 1, :], start=True, stop=True)
                rbiA = work.tile([HS, 512], F32, tag="rbiA", name="rbiA")
                nc.vector.reciprocal_approx_fast(out=rbiA[:], in_=rb[0:HS, 0:512])

                stB = work.tile([1, 512], F32, tag="stB", name="stB")
                nc.vector.tensor_copy(out=stB[:], in_=ypsB[0:1, :])
                rtB = work.tile([1, 512], F32, tag="rtB", name="rtB")
                nc.vector.reciprocal_approx_fast(out=rtB[:], in_=stB[:])
                rtBb = work.tile([1, 512], MM, tag="rtBb", name="rtBb")
                with nc.allow_low_precision(reason="softmax denom staged bf16"):
                    nc.vector.tensor_copy(out=rtBb[:], in_=rtB[:])
                nc.tensor.matmul(rb[HS:P, 512:1024], ones65[0:1, :],
                                 rtBb[:], start=True, stop=True,
                                 tile_position=(0, HS))
                rbiB = work.tile([P, 512], F32, tag="rbiB", name="rbiB")
                nc.vector.tensor_copy(out=rbiB[HS:P, :], in_=rb[HS:P, 512:1024])

                yt = ytp.tile([P, 512], MM, tag=f"ytp{hp}", name=f"ytp{hp}")
                nc.vector.tensor_mul(out=yt[0:HS, :], in0=ypsA[0:HS, :],
                                     in1=rbiA[:])
                nc.vector.tensor_mul(out=yt[HS:P, :], in0=ypsB[HS:P, :],
                                     in1=rbiB[HS:P, :])
                return yt

            for c in range(NQCH):
                ytiles = [None] * 3
                pend = None
                jlast = 4 * c + 3
                for hp in range(3):
                    qTA = qkT[hp][0:HS, :]
                    qTB = qkT[hp][HS:P, :]
                    kTA = qkT[3 + hp][0:HS, :]
                    kTB = qkT[3 + hp][HS:P, :]
                    vp = [
                        vsb[j].rearrange("p (b e) -> p b e", e=VPB)[:, hp, :]
                        for j in range(4 * c + 4)
                    ]

                    ypsA = psY.tile([HS + 1, 512], F32, tag="y", name="ypsA")
                    ypsB = psY.tile([P, 512], F32, tag="y", name="ypsB")
                    for j in range(4 * c + 4):
                        m = j - 4 * c
                        qs = P * m if m > 0 else 0
                        sps = psMM.tile([P, 1024], F32, tag="mm", name="ps_s")
                        es = work.tile([P, 1024], MM, tag="es", name="es")
                        # both heads' S blocks, row-tiled (A rows 0-63, B 64-127)
                        nc.tensor.matmul(
                            sps[:, qs:512],
                            kTA[:, P * j:P * (j + 1)],
                            qTA[:, 512 * c + qs:512 * (c + 1)],
                            start=True, stop=True,
                        )
                        nc.tensor.matmul(
                            sps[:, 512 + qs:1024],
                            kTB[:, P * j:P * (j + 1)],
                            qTB[:, 512 * c + qs:512 * (c + 1)],
                            start=True, stop=True,
                        )
                        if qs > 0:
                            nc.scalar.activation(
                                out=es[:, qs:512], in_=sps[:, qs:512],
                                func=mybir.ActivationFunctionType.Exp,
                                scale=1.0 / 8.0)
                            nc.scalar.activation(
                                out=es[:, 512 + qs:1024], in_=sps[:, 512 + qs:1024],
                                func=mybir.ActivationFunctionType.Exp,
                                scale=1.0 / 8.0)
                        else:
                            nc.scalar.activation(
                                out=es[:], in_=sps[:],
                                func=mybir.ActivationFunctionType.Exp,
                                scale=1.0 / 8.0)
                        if m >= 0:
                            nc.vector.tensor_mul(
                                out=es[:, qs:qs + P],
                                in0=es[:, qs:qs + P], in1=mask_sb[:])
                            nc.vector.tensor_mul(
                                out=es[:, 512 + qs:512 + qs + P],
                                in0=es[:, 512 + qs:512 + qs + P], in1=mask_sb[:])
                        nc.tensor.matmul(
                            ypsA[:, qs:512], vp[j][:, 0:HS + 1],
                            es[:, qs:512],
                            start=(j == 0), stop=(j == jlast),
                        )
                        nc.tensor.matmul(
                            ypsB[:, qs:512], vp[j][:, HS:VPB],
                            es[:, 512 + qs:1024],
                            start=(j == 0), stop=(j == jlast),
                        )

                    if pend is not None:
                        php, pA, pB = pend
                        ytiles[php] = emit_norm_pair(php, pA, pB)
                    pend = (hp, ypsA, ypsB)

                php, pA, pB = pend
                ytiles[php] = emit_norm_pair(php, pA, pB)

                # projection for the 4 token-blocks of this chunk (K=128)
                for tq in range(4):
                    tb = 4 * c + tq
                    pps = psMM.tile([P, 1024], F32, tag="mm", name="ps_o")
                    for hp in range(3):
                        for n0, nn in ((0, 512), (512, 256)):
                            nc.tensor.matmul(
                                pps[:, n0:n0 + nn],
                                ytiles[hp][:, P * tq:P * (tq + 1)],
                                wph[hp][:, n0:n0 + nn],
                                start=(hp == 0),
                                stop=(hp == 2),
                            )
                    ot = work.tile([P, C], F32, tag="ot", name="ot")
                    nc.vector.tensor_copy(out=ot[:], in_=pps[:, 0:C])
                    nc.sync.dma_start(outv[tb], ot[:])

    nc.compile()
    return nc


_nc_cache = None
last_results = None


def _get_nc():
    global _nc_cache
    if _nc_cache is None:
        _nc_cache = _build_nc()
    return _nc_cache


def make_in_maps(x, W_attn, b_attn, W_proj):
    x = np.asarray(x, np.float32)
    W_attn = np.asarray(W_attn, np.float32)
    b_attn = np.asarray(b_attn, np.float32)
    W_proj = np.asarray(W_proj, np.float32)

    kk, qq = np.meshgrid(np.arange(P), np.arange(P), indexing="ij")
    mask = (qq >= kk).astype(NP_MM)

    in_maps = []
    for core in range(NCORES):
        b, g = divmod(core, 2)
        hs = slice(GC * g, GC * (g + 1))
        bq = b_attn[0:C][hs]
        bk = b_attn[C:2 * C][hs]
        bvs = b_attn[2 * C:3 * C][hs]
        bqk = np.stack(
            [bq[P * p:P * (p + 1)] for p in range(3)]
            + [bk[P * p:P * (p + 1)] for p in range(3)],
            axis=1,
        ).astype(np.float32)
        in_maps.append({
            "xT": np.ascontiguousarray(x[b].T).astype(NP_MM),
            "wq": np.ascontiguousarray(W_attn[:, 0:C][:, hs]).astype(NP_MM),
            "wk": np.ascontiguousarray(W_attn[:, C:2 * C][:, hs]).astype(NP_MM),
            "wv": np.ascontiguousarray(W_attn[:, 2 * C:3 * C][:, hs]).astype(NP_MM),
            "wp": np.ascontiguousarray(W_proj[hs, :]).astype(NP_MM),
            "bqk": np.ascontiguousarray(bqk),
            "bv": np.ascontiguousarray(bvs.reshape(1, GC)),
            "mask": mask,
        })
    return in_maps


def kernel(x, W_attn, b_attn, W_proj, b_proj, _trace=False):
    global last_results
    nc = _get_nc()
    in_maps = make_in_maps(x, W_attn, b_attn, W_proj)
    res = run_bass_kernel_spmd(nc, in_maps, list(range(NCORES)), trace=_trace)
    last_results = res
    out = np.zeros((B, T, C), np.float32)
    for core in range(NCORES):
        out[core // 2] += res.results[core]["out"]
    out += np.asarray(b_proj, np.float32)[None, None, :]
    return out
